# revision 6
# baseline (speedup 1.0000x reference)
"""PRMPConv GNN message-passing kernel for 8 Trainium2 NeuronCores.

Strategy: sort edges by dst, partition dst-node windows (128 dsts each)
contiguously across 8 cores (balanced by edge count). Each core:
  - computes P = MLP(x_dst) for its windows (per-window matmuls)
  - gathers x_src rows per edge (indirect DMA, 128 rows/instr)
  - d = x_j - P[dst] built in PSUM via one-hot expansion matmul + identity
    inject matmul
  - LN stats via ACT accumulate, z = (d-mu)*istd folded into the
    segment-sum matmul (S' = one-hot * istd, plus -mu correction matmul)
  - per-window aggr^T accumulated in PSUM, scaled by 1/cnt, fed to the
    final update linear (weights/biases preprocessed on host: gamma/beta
    folded in)
Output assembled host-side from per-core [128, W*128] transposed blocks.
"""
import sys

sys.path.insert(0, "/opt/trn_rl_repo")

import numpy as np
from contextlib import ExitStack

import concourse.bass as bass
import concourse.bacc as bacc
import concourse.tile as tile
from concourse import mybir
from concourse.bass_utils import run_bass_kernel_spmd

N_SRC, N_DST, E, DIM = 50000, 50000, 800000, 128
P = 128
N_CORES = 8
LN_EPS = 1e-5
F32 = mybir.dt.float32
F32R = mybir.dt.float32r
I32 = mybir.dt.int32
AF = mybir.ActivationFunctionType
ALU = mybir.AluOpType


def _host_prep(x_src, x_dst, edge_index):
    src = np.asarray(edge_index[0], dtype=np.int64)
    dst = np.asarray(edge_index[1], dtype=np.int64)

    order = np.argsort(dst, kind="stable")
    src_s = src[order].astype(np.int32)
    dst_s = dst[order].astype(np.int32)

    n_win = (N_DST + P - 1) // P  # 391
    cnt = np.bincount(dst_s, minlength=n_win * P).astype(np.float32)

    # edges per window, then contiguous window->core split balanced by edges
    win_of_edge = dst_s // P
    edges_per_win = np.bincount(win_of_edge, minlength=n_win)
    cum = np.cumsum(edges_per_win)
    bounds = [0]
    for c in range(1, N_CORES):
        target = E * c // N_CORES
        bounds.append(int(np.searchsorted(cum, target)))
    bounds.append(n_win)
    bounds = np.array(bounds)
    w_count = bounds[1:] - bounds[:-1]
    W = int(w_count.max())

    T_w = int(np.ceil(edges_per_win.max() / P))  # tiles per window (uniform)
    NT = W * T_w

    # edge start offsets per window in the sorted stream
    win_start = np.concatenate([[0], cum])

    src_cols = np.zeros((N_CORES, P, NT), np.int32)        # gather indices (col t)
    slot_cols = np.full((N_CORES, P, NT), -1.0, np.float32)  # slot ids (col t)
    slot_rows = np.full((N_CORES, W, T_w * P), -1.0, np.float32)
    invcnt = np.ones((N_CORES, W, P), np.float32)
    has_edge = np.zeros((N_CORES, W, P), np.float32)
    x_dst_pad = np.zeros((N_CORES, W * P, DIM), np.float32)

    for c in range(N_CORES):
        for wi, w in enumerate(range(bounds[c], bounds[c + 1])):
            s, e = win_start[w], win_start[w + 1]
            n_e = e - s
            esrc = src_s[s:e]
            eslot = (dst_s[s:e] - w * P).astype(np.float32)
            pad = T_w * P - n_e
            esrc = np.concatenate([esrc, np.zeros(pad, np.int32)])
            eslot = np.concatenate([eslot, np.full(pad, -1.0, np.float32)])
            # tile t edge p = stream position t*P + p
            src_cols[c, :, wi * T_w:(wi + 1) * T_w] = esrc.reshape(T_w, P).T
            slot_cols[c, :, wi * T_w:(wi + 1) * T_w] = eslot.reshape(T_w, P).T
            slot_rows[c, wi, :] = eslot
            node_lo = w * P
            node_hi = min((w + 1) * P, N_DST)
            nv = node_hi - node_lo
            cw = cnt[node_lo:node_lo + P]
            invcnt[c, wi, :] = 1.0 / np.maximum(cw, 1.0)
            has_edge[c, wi, :] = (cw > 0).astype(np.float32)
            x_dst_pad[c, wi * P:wi * P + nv, :] = x_dst[node_lo:node_hi]

    meta = dict(W=W, T_w=T_w, NT=NT, bounds=bounds)
    arrays = dict(
        src_cols=src_cols, slot_cols=slot_cols, slot_rows=slot_rows,
        invcnt=invcnt, has_edge=has_edge, x_dst_pad=x_dst_pad,
    )
    return meta, arrays


def _build_program(W, T_w, NT):
    nc = bacc.Bacc("TRN2", target_bir_lowering=False, debug=False,
                   num_devices=N_CORES)

    t_xsrc = nc.dram_tensor("xsrc", [N_SRC, DIM], F32R, kind="ExternalInput")
    t_xdst = nc.dram_tensor("xdst", [W * P, DIM], F32, kind="ExternalInput")
    t_srcix = nc.dram_tensor("srcix", [P, NT], I32, kind="ExternalInput")
    t_slotc = nc.dram_tensor("slotc", [P, NT], F32, kind="ExternalInput")
    t_slotr = nc.dram_tensor("slotr", [W, T_w * P], F32R, kind="ExternalInput")
    t_invc = nc.dram_tensor("invc", [W, P], F32, kind="ExternalInput")
    t_hedge = nc.dram_tensor("hedge", [W, P], F32R, kind="ExternalInput")
    t_w1 = nc.dram_tensor("w1", [DIM, DIM], F32R, kind="ExternalInput")
    t_w2 = nc.dram_tensor("w2", [DIM, DIM], F32R, kind="ExternalInput")
    t_wut = nc.dram_tensor("wut", [DIM, DIM], F32R, kind="ExternalInput")
    t_wub = nc.dram_tensor("wub", [DIM, DIM], F32R, kind="ExternalInput")
    t_b1 = nc.dram_tensor("b1c", [DIM, 1], F32, kind="ExternalInput")
    t_nb2 = nc.dram_tensor("nb2c", [DIM, 1], F32, kind="ExternalInput")
    t_bu = nc.dram_tensor("buc", [DIM, 1], F32, kind="ExternalInput")
    t_wbb = nc.dram_tensor("wbb", [1, DIM], F32R, kind="ExternalInput")
    t_iotar = nc.dram_tensor("iotar", [P, P], F32, kind="ExternalInput")
    t_iotac = nc.dram_tensor("iotac", [P, 1], F32, kind="ExternalInput")
    t_ones = nc.dram_tensor("ones", [1, P], F32R, kind="ExternalInput")
    t_ident = nc.dram_tensor("ident", [P, P], F32R, kind="ExternalInput")
    t_out = nc.dram_tensor("out", [P, W * P], F32, kind="ExternalOutput")

    with tile.TileContext(nc) as tc, ExitStack() as ctx:
        const = ctx.enter_context(tc.tile_pool(name="const", bufs=1))
        keep = ctx.enter_context(tc.tile_pool(name="keep", bufs=1))
        stage = ctx.enter_context(tc.tile_pool(name="stage", bufs=3))
        gat = ctx.enter_context(tc.tile_pool(name="gat", bufs=3))
        stp = ctx.enter_context(tc.tile_pool(name="stp", bufs=3))
        dpool = ctx.enter_context(tc.tile_pool(name="dpool", bufs=2))
        wpost = ctx.enter_context(tc.tile_pool(name="wpost", bufs=3))
        p_rep = ctx.enter_context(tc.tile_pool(name="p_rep", bufs=2, space="PSUM"))
        p_d = ctx.enter_context(tc.tile_pool(name="p_d", bufs=2, space="PSUM"))
        p_agg = ctx.enter_context(tc.tile_pool(name="p_agg", bufs=2, space="PSUM"))
        p_st = ctx.enter_context(tc.tile_pool(name="p_st", bufs=2, space="PSUM"))

        # constants
        iota_r = const.tile([P, P], F32)
        iota_c = const.tile([P, 1], F32)
        ones_r = const.tile([1, P], F32R)
        ident = const.tile([P, P], F32R)
        identf = const.tile([P, P], F32)
        w1 = const.tile([DIM, DIM], F32R)
        w2 = const.tile([DIM, DIM], F32R)
        wut = const.tile([DIM, DIM], F32R)
        wub = const.tile([DIM, DIM], F32R)
        b1c = const.tile([DIM, 1], F32)
        nb2c = const.tile([DIM, 1], F32)
        buc = const.tile([DIM, 1], F32)
        wbb = const.tile([1, DIM], F32R)
        eps_c = const.tile([P, 1], F32)
        nc.vector.memset(eps_c[:], LN_EPS)
        nc.sync.dma_start(out=iota_r[:], in_=t_iotar[:, :])
        nc.sync.dma_start(out=iota_c[:], in_=t_iotac[:, :])
        nc.sync.dma_start(out=ones_r[:], in_=t_ones[:, :])
        nc.sync.dma_start(out=ident[:], in_=t_ident[:, :])
        nc.sync.dma_start(out=identf[:], in_=t_ident[:, :].bitcast(F32))
        nc.sync.dma_start(out=w1[:], in_=t_w1[:, :])
        nc.sync.dma_start(out=w2[:], in_=t_w2[:, :])
        nc.sync.dma_start(out=wut[:], in_=t_wut[:, :])
        nc.sync.dma_start(out=wub[:], in_=t_wub[:, :])
        nc.sync.dma_start(out=b1c[:], in_=t_b1[:, :])
        nc.sync.dma_start(out=nb2c[:], in_=t_nb2[:, :])
        nc.sync.dma_start(out=buc[:], in_=t_bu[:, :])
        nc.sync.dma_start(out=wbb[:], in_=t_wbb[:, :])

        # all gather indices, strided column slices used per tile
        srcix = keep.tile([P, NT], I32)
        slotc = keep.tile([P, NT], F32)
        nc.sync.dma_start(out=srcix[:], in_=t_srcix[:, :])
        nc.sync.dma_start(out=slotc[:], in_=t_slotc[:, :])

        for w in range(W):
            # ---- stage A: per-window MLP -> negP_win [nodes, dims] ----
            xd = stage.tile([P, DIM], F32)
            nc.sync.dma_start(out=xd[:], in_=t_xdst[w * P:(w + 1) * P, :])
            xdt_ps = p_st.tile([P, P], F32, space="PSUM", name="st_ps")
            nc.tensor.transpose(out=xdt_ps[:], in_=xd[:], identity=identf[:])
            xdt = stage.tile([P, P], F32R, name="xdt")
            nc.scalar.copy(out=xdt[:], in_=xdt_ps[:])
            h1_ps = p_st.tile([P, P], F32, space="PSUM", name="st_ps")
            nc.tensor.matmul(out=h1_ps[:], lhsT=w1[:], rhs=xdt[:], start=True, stop=True)
            h1t = stage.tile([P, P], F32R)
            nc.scalar.activation(out=h1t[:], in_=h1_ps[:], func=AF.Relu, bias=b1c[:])
            pt_ps = p_st.tile([P, P], F32, space="PSUM", name="st_ps")
            nc.tensor.matmul(out=pt_ps[:], lhsT=w2[:], rhs=h1t[:], start=True, stop=True)
            negpt = stage.tile([P, P], F32)
            nc.scalar.activation(out=negpt[:], in_=pt_ps[:], func=AF.Identity,
                                 bias=nb2c[:], scale=-1.0)
            npw_ps = p_st.tile([P, P], F32, space="PSUM", name="st_ps")
            nc.tensor.transpose(out=npw_ps[:], in_=negpt[:], identity=identf[:])
            negpw = stage.tile([P, P], F32R)
            nc.scalar.copy(out=negpw[:], in_=npw_ps[:])

            slotr = stage.tile([1, T_w * P], F32R)
            nc.sync.dma_start(out=slotr[:], in_=t_slotr[w, None, :])

            # ---- edge tiles: gather + d in PSUM + stats ----
            stats_sum = stp.tile([P, T_w], F32)
            stats_sq = stp.tile([P, T_w], F32)
            d_tiles = []
            for t in range(T_w):
                rep_ps = p_rep.tile([P, P], F32, space="PSUM")
                nc.tensor.matmul(out=rep_ps[:], lhsT=ones_r[:],
                                 rhs=slotr[:, t * P:(t + 1) * P],
                                 start=True, stop=True)
                s_t = gat.tile([P, P], F32R, name=f"sT_{t % 4}")
                nc.vector.tensor_tensor(out=s_t[:], in0=rep_ps[:],
                                        in1=iota_c[:].to_broadcast([P, P]),
                                        op=ALU.is_equal)
                xg = gat.tile([P, DIM], F32R, name=f"xg_{t % 4}")
                nc.gpsimd.indirect_dma_start(
                    out=xg[:], out_offset=None, in_=t_xsrc[:],
                    in_offset=bass.IndirectOffsetOnAxis(
                        ap=srcix[:, w * T_w + t:w * T_w + t + 1], axis=0),
                )
                d_ps = p_d.tile([P, DIM], F32, space="PSUM")
                nc.tensor.matmul(out=d_ps[:], lhsT=s_t[:], rhs=negpw[:],
                                 start=True, stop=False)
                nc.tensor.matmul(out=d_ps[:], lhsT=ident[:], rhs=xg[:],
                                 start=False, stop=True)
                d_sb = dpool.tile([P, DIM], F32R, name=f"d_{t}")
                nc.scalar.activation(out=d_sb[:], in_=d_ps[:], func=AF.Copy,
                                     accum_out=stats_sum[:, t:t + 1])
                sq_scr = gat.tile([P, DIM], F32, name=f"sq_{t % 4}")
                nc.scalar.activation(out=sq_scr[:], in_=d_ps[:], func=AF.Square,
                                     accum_out=stats_sq[:, t:t + 1])
                d_tiles.append(d_sb)

            # ---- window stats: istd, negmu ----
            mu2 = stp.tile([P, T_w], F32)
            nc.vector.scalar_tensor_tensor(out=mu2[:], in0=stats_sum[:],
                                           scalar=1.0 / (DIM * DIM),
                                           in1=stats_sum[:],
                                           op0=ALU.mult, op1=ALU.mult)
            var = stp.tile([P, T_w], F32)
            nc.vector.scalar_tensor_tensor(out=var[:], in0=stats_sq[:],
                                           scalar=1.0 / DIM, in1=mu2[:],
                                           op0=ALU.mult, op1=ALU.subtract)
            sd = stp.tile([P, T_w], F32)
            nc.scalar.activation(out=sd[:], in_=var[:], func=AF.Sqrt, bias=eps_c[:])
            istd = stp.tile([P, T_w], F32)
            nc.vector.reciprocal(out=istd[:], in_=sd[:])
            negmu = stp.tile([P, T_w], F32R)
            nc.vector.tensor_scalar_mul(out=negmu[:], in0=stats_sum[:],
                                        scalar1=-1.0 / DIM)

            # ---- second pass: S' build + segment matmuls ----
            agg_ps = p_agg.tile([P, P], F32, space="PSUM")
            for t in range(T_w):
                sp = gat.tile([P, P], F32R, name=f"sp_{t % 4}")
                nc.vector.scalar_tensor_tensor(
                    out=sp[:], in0=iota_r[:], scalar=slotc[:, w * T_w + t:w * T_w + t + 1],
                    in1=istd[:, t:t + 1].to_broadcast([P, P]),
                    op0=ALU.is_equal, op1=ALU.mult)
                nc.tensor.matmul(out=agg_ps[:], lhsT=d_tiles[t][:], rhs=sp[:],
                                 start=(t == 0), stop=False)
                nc.tensor.matmul(out=agg_ps[:],
                                 lhsT=negmu[:, t:t + 1].to_broadcast([P, P]),
                                 rhs=sp[:], start=False, stop=(t == T_w - 1))

            # ---- window post: scale by invcnt, final linear ----
            invr = wpost.tile([1, P], F32R)
            nc.sync.dma_start(out=invr[:], in_=t_invc[w, None, :].bitcast(F32R))
            hr = wpost.tile([1, P], F32R)
            nc.sync.dma_start(out=hr[:], in_=t_hedge[w, None, :])
            ic_ps = p_rep.tile([P, P], F32, space="PSUM", name="rep_ps")
            nc.tensor.matmul(out=ic_ps[:], lhsT=ones_r[:], rhs=invr[:],
                             start=True, stop=True)
            ic_sb = wpost.tile([P, P], F32)
            nc.scalar.copy(out=ic_sb[:], in_=ic_ps[:])
            m_sb = wpost.tile([P, P], F32R)
            nc.vector.tensor_tensor(out=m_sb[:], in0=agg_ps[:], in1=ic_sb[:],
                                    op=ALU.mult)
            out_ps = p_agg.tile([P, P], F32, space="PSUM", name="agg_ps")
            nc.tensor.matmul(out=out_ps[:], lhsT=wut[:], rhs=xdt[:],
                             start=True, stop=False)
            nc.tensor.matmul(out=out_ps[:], lhsT=wub[:], rhs=m_sb[:],
                             start=False, stop=False)
            nc.tensor.matmul(out=out_ps[:], lhsT=wbb[:], rhs=hr[:],
                             start=False, stop=True)
            out_sb = wpost.tile([P, P], F32)
            nc.scalar.activation(out=out_sb[:], in_=out_ps[:], func=AF.Identity,
                                 bias=buc[:])
            nc.sync.dma_start(out=t_out[:, w * P:(w + 1) * P], in_=out_sb[:])

    nc.compile()
    return nc


LAST_EXEC_NS = None
LAST_RESULTS = None


def kernel(x_src, x_dst, edge_index, W1, b1, W2, b2, gamma, beta, Wu, bu,
           _trace=False, _tmpdir=None):
    global LAST_EXEC_NS, LAST_RESULTS
    x_src = np.ascontiguousarray(np.asarray(x_src, dtype=np.float32))
    x_dst = np.ascontiguousarray(np.asarray(x_dst, dtype=np.float32))
    W1 = np.asarray(W1, np.float32); b1 = np.asarray(b1, np.float32)
    W2 = np.asarray(W2, np.float32); b2 = np.asarray(b2, np.float32)
    gamma = np.asarray(gamma, np.float32); beta = np.asarray(beta, np.float32)
    Wu = np.asarray(Wu, np.float32); bu = np.asarray(bu, np.float32)

    meta, arr = _host_prep(x_src, x_dst, edge_index)
    W, T_w, NT = meta["W"], meta["T_w"], meta["NT"]
    bounds = meta["bounds"]

    nc = _build_program(W, T_w, NT)

    wu_top = np.ascontiguousarray(Wu[:DIM, :])
    wu_bot = np.ascontiguousarray(Wu[DIM:, :] * gamma[:, None])
    wbb = (np.asarray(Wu[DIM:, :], np.float64).T @ beta).astype(np.float32)

    iota_r = np.tile(np.arange(P, dtype=np.float32)[None, :], (P, 1))
    iota_c = np.arange(P, dtype=np.float32)[:, None]
    ones_r = np.ones((1, P), np.float32)
    ident = np.eye(P, dtype=np.float32)

    in_maps = []
    for c in range(N_CORES):
        in_maps.append({
            "xsrc": x_src, "xdst": arr["x_dst_pad"][c],
            "srcix": arr["src_cols"][c], "slotc": arr["slot_cols"][c],
            "slotr": arr["slot_rows"][c], "invc": arr["invcnt"][c],
            "hedge": arr["has_edge"][c],
            "w1": W1, "w2": W2, "wut": wu_top, "wub": wu_bot,
            "b1c": b1[:, None], "nb2c": -b2[:, None], "buc": bu[:, None],
            "wbb": wbb[None, :],
            "iotar": iota_r, "iotac": iota_c, "ones": ones_r, "ident": ident,
        })

    res = run_bass_kernel_spmd(nc, in_maps, core_ids=list(range(N_CORES)),
                               trace=_trace, tmpdir=_tmpdir)
    LAST_EXEC_NS = res.exec_time_ns
    LAST_RESULTS = res

    out = np.zeros((N_DST + P, DIM), np.float32)
    for c in range(N_CORES):
        blk = res.results[c]["out"]  # [P, W*P] transposed
        n_nodes = (bounds[c + 1] - bounds[c]) * P
        node_lo = bounds[c] * P
        out[node_lo:node_lo + n_nodes] = blk.T[:n_nodes]
    return out[:N_DST]


def _reference_np(x_src, x_dst, edge_index, W1, b1, W2, b2, gamma, beta, Wu, bu):
    src_idx, dst_idx = edge_index[0], edge_index[1]
    x_j = x_src[src_idx]
    h = x_dst[dst_idx]
    predicted = np.maximum(h @ W1 + b1, 0.0) @ W2 + b2
    d = x_j - predicted
    mu = d.mean(-1, keepdims=True)
    var = ((d - mu) ** 2).mean(-1, keepdims=True)
    residual = (d - mu) / np.sqrt(var + 1e-5) * gamma + beta
    aggr = np.zeros((x_dst.shape[0], d.shape[1]), np.float64)
    np.add.at(aggr, dst_idx, residual)
    cnt = np.bincount(dst_idx, minlength=x_dst.shape[0]).astype(np.float64)
    aggr = aggr / np.maximum(cnt, 1.0)[:, None]
    return np.concatenate([x_dst, aggr], -1) @ Wu + bu


if __name__ == "__main__":
    sys.path.insert(0, "/root/problem")
    import reference

    inputs = {k: np.asarray(v) for k, v in reference.setup_inputs().items()}
    got = kernel(**inputs)
    exp = _reference_np(**{k: (np.asarray(v) if np.issubdtype(np.asarray(v).dtype, np.integer)
                           else np.asarray(v, np.float64)) for k, v in inputs.items()})
    err = np.abs(got - exp).max() / np.abs(exp).max()
    print(f"Relative error: {err:.3e}")
